# revision 1
# baseline (speedup 1.0000x reference)
"""LlamaAttention (B=2, S=2048, H=4096, 32 q heads / 8 kv heads, RoPE, causal)
on 8 Trainium2 NeuronCores.

Sharding: data-parallel over batch (2) x tensor-parallel over heads (4).
Core c = b*4 + t handles batch b with q heads 8t..8t+7 and kv heads 2t..2t+1.
Each core computes a partial output y_c = attn_out_local @ wo_local^T
([S, H], fp32); the host sums the 4 TP partials per batch.

All matmuls bf16 inputs / fp32 PSUM accumulation. All weight/activation
DRAM tensors are pre-interleaved on the host so each DMA is a direct image
of its SBUF destination (>=8KB contiguous per partition -> near-peak DMA).

Per-core structure (v3 - attention interleaved under Q projection so the
ScalarE exp work hides beneath TensorE matmuls):
  KV phase : kT[d, s] per kv head (+ fused RoPE); v in natural [s, d] layout
             with a ones column (vAug) so the softmax denominator falls out
             of the AV matmul for free.
  QA phase : per 512-token block tb: Q-proj for 8 heads (two 4-head PSUM
             half-passes over one resident hs tile) + RoPE -> qTb; then
             causal attention for all heads at q-block tb: scores computed
             transposed (sT[k, q] = kT_chunk.T @ qTb) -> exp (no
             max-subtraction; scores are O(1)) -> multiplicative bf16 0/1
             diagonal masks -> AV with vAug -> normalize by the ones column
             -> PE transpose -> oTb -> spilled to DRAM scratch.
  O phase  : y[s, :] accumulated over the 8 local head-dim chunks from the
             DRAM oT scratch, stored via [128, H] fp32 row buffers.
"""
import sys

sys.path.insert(0, "/opt/trn_rl_repo")

import numpy as np
import ml_dtypes

BF16 = ml_dtypes.bfloat16

B, S, H = 2, 2048, 4096
NH, NKV, HD = 32, 8, 128
THETA = 10000.0
SCALE = 1.0 / float(np.sqrt(HD))

N_CORES = 8
TP = 4
NH_L = NH // TP        # 8 local q heads
NKV_L = NKV // TP      # 2 local kv heads
GRP_L = NH_L // NKV_L  # 4 q heads per local kv head
TOKB = 512
NKC = H // 128         # 32 contraction chunks
NTB = S // TOKB        # 4 token blocks
NQC = S // 128         # 16 token chunks
VSTRIDE = 132          # per-chunk stride in vAug (129 used, pad for alignment)
KHALF = NKC // 2

_NC_CACHE = {}


def _rope(nc, rp, psum, cos_sb, sinn_sb, tsl, outT, col0, f32):
    """RoPE on a [128(d), TOKB] fp32 PSUM block; writes bf16 to outT[:, col0:+TOKB].

    out[0:64]   = p[0:64]*cos - p[64:128]*sin
    out[64:128] = p[64:128]*cos + p[0:64]*sin
    (cos rows duplicated; sinn rows 0:64 pre-negated on host; fp16 tables.)
    """
    tcos = rp.tile([128, TOKB], f32, tag="tcos")
    nc.vector.tensor_mul(tcos[:], psum[:], cos_sb[:, tsl])
    trs = rp.tile([128, TOKB], f32, tag="trs")
    nc.vector.tensor_mul(trs[0:64, :], psum[64:128, :], sinn_sb[0:64, tsl])
    nc.vector.tensor_mul(trs[64:128, :], psum[0:64, :], sinn_sb[64:128, tsl])
    nc.vector.tensor_add(outT[:, col0: col0 + TOKB], tcos[:], trs[:])


def _build(reps=1, phases="full"):
    import concourse.mybir as mybir
    import concourse.tile as tile
    from concourse import bacc
    from contextlib import ExitStack

    dt = mybir.dt
    f32, bf16, f16 = dt.float32, dt.bfloat16, dt.float16
    af = mybir.ActivationFunctionType

    nc = bacc.Bacc("TRN2", target_bir_lowering=False, debug=False,
                   enable_asserts=True, num_devices=N_CORES)
    # all pre-interleaved on host: DMA source rows == SBUF partition images
    hs_d = nc.dram_tensor("hs", [NTB * 128, NKC * TOKB], bf16, kind="ExternalInput").ap()
    wq_d = nc.dram_tensor("wq", [128, NKC * NH_L * 128], bf16, kind="ExternalInput").ap()
    wk_d = nc.dram_tensor("wk", [128, NKC * NKV_L * 128], bf16, kind="ExternalInput").ap()
    wv_d = nc.dram_tensor("wv", [128, NKC * NKV_L * 128], bf16, kind="ExternalInput").ap()
    wo_d = nc.dram_tensor("wo", [128, NH_L * H], bf16, kind="ExternalInput").ap()
    cos_d = nc.dram_tensor("cosT", [128, S], f16, kind="ExternalInput").ap()
    sin_d = nc.dram_tensor("sinN", [128, S], f16, kind="ExternalInput").ap()
    msk_d = nc.dram_tensor("maskB", [128, 4 * TOKB], bf16, kind="ExternalInput").ap()
    id_d = nc.dram_tensor("ident", [128, 128], bf16, kind="ExternalInput").ap()
    y_d = nc.dram_tensor("y", [S, H], f32, kind="ExternalOutput").ap()

    def emit(ctx, tc):
        ps = ctx.enter_context(tc.tile_pool(name="ps", bufs=8, space="PSUM"))
        persist = ctx.enter_context(tc.tile_pool(name="persist", bufs=1))
        dram = ctx.enter_context(tc.tile_pool(name="dram", bufs=1, space="DRAM"))

        mask_sb = persist.tile([128, 4 * TOKB], bf16, tag="mask")
        nc.sync.dma_start(mask_sb[:], msk_d[:])
        id_sb = persist.tile([128, 128], bf16, tag="ident")
        nc.sync.dma_start(id_sb[:], id_d[:])
        kT = persist.tile([128, NKV_L * S], bf16, tag="kT")
        vA = persist.tile([128, NKV_L * NQC * VSTRIDE], bf16, tag="vA")
        nc.gpsimd.memset(vA[:], 1.0)
        # oT scratch in HBM: col (tb*4096 + h*512 + s2*128)
        oT_dram = dram.tile([128, NH_L * S], bf16, tag="oTd")

        with tc.tile_pool(name="cs", bufs=1) as csp, \
             tc.tile_pool(name="rope", bufs=2) as rp, \
             tc.tile_pool(name="wq", bufs=1) as wqp:
            cos_sb = csp.tile([128, S], f16, tag="cos")
            nc.sync.dma_start(cos_sb[:], cos_d[:])
            sinn_sb = csp.tile([128, S], f16, tag="sinn")
            nc.sync.dma_start(sinn_sb[:], sin_d[:])
            wq_sb = wqp.tile([128, NKC * NH_L * 128], bf16, tag="wq")
            nc.sync.dma_start(wq_sb[:], wq_d[:])

            if phases in ("full", "kv", "kvqa"):
                emit_kv(tc, ps, rp, cos_sb, sinn_sb, kT, vA)
            if phases == "qa":
                nc.vector.memset(kT[:], 0.01)
            if phases in ("full", "qa", "kvqa", "kvq"):
                emit_qa(tc, ps, rp, cos_sb, sinn_sb, kT, vA,
                        mask_sb, id_sb, oT_dram, wq_sb,
                        skip_attn=(phases == "kvq"))
        if phases in ("full", "o"):
            emit_o(tc, ps, oT_dram)

    def emit_kv(tc, ps, rp, cos_sb, sinn_sb, kT, vA):
        with tc.tile_pool(name="wkv", bufs=1) as wkvp, \
             tc.tile_pool(name="hskv", bufs=3) as hsp:
            wk_sb = wkvp.tile([128, NKC * NKV_L * 128], bf16, tag="wk")
            nc.sync.dma_start(wk_sb[:], wk_d[:])
            wv_sb = wkvp.tile([128, NKC * NKV_L * 128], bf16, tag="wv")
            nc.sync.dma_start(wv_sb[:], wv_d[:])
            for tb in range(NTB):
                tsl = slice(tb * TOKB, (tb + 1) * TOKB)
                pks = [ps.tile([128, TOKB], f32, tag="ps", name=f"pk{tb}_{i}")
                       for i in range(NKV_L)]
                pvs = [ps.tile([128, 256], f32, tag="ps", name=f"pv{tb}_{i}")
                       for i in range(4)]
                for kh in range(2):
                    ht = hsp.tile([128, KHALF * TOKB], bf16, tag="hs",
                                  name=f"hskv{tb}_{kh}")
                    nc.sync.dma_start(
                        ht[:], hs_d[tb * 128:(tb + 1) * 128,
                                    kh * KHALF * TOKB:(kh + 1) * KHALF * TOKB])
                    for k2 in range(KHALF):
                        kc = kh * KHALF + k2
                        hsl = slice(k2 * TOKB, (k2 + 1) * TOKB)
                        for g in range(NKV_L):
                            c0 = kc * 256 + g * 128
                            nc.tensor.matmul(
                                pks[g][:], wk_sb[:, c0:c0 + 128], ht[:, hsl],
                                start=(kc == 0), stop=(kc == NKC - 1))
                        for s in range(4):
                            nc.tensor.matmul(
                                pvs[s][:],
                                ht[:, k2 * TOKB + s * 128: k2 * TOKB + (s + 1) * 128],
                                wv_sb[:, kc * 256:(kc + 1) * 256],
                                start=(kc == 0), stop=(kc == NKC - 1))
                for g in range(NKV_L):
                    _rope(nc, rp, pks[g], cos_sb, sinn_sb, tsl, kT, g * S + tb * TOKB, f32)
                for s in range(4):
                    qc = tb * 4 + s
                    for g in range(NKV_L):
                        c0 = (g * NQC + qc) * VSTRIDE
                        nc.vector.tensor_copy(
                            vA[:, c0:c0 + 128],
                            pvs[s][:, g * 128:(g + 1) * 128])

    def emit_qa(tc, ps, rp, cos_sb, sinn_sb, kT, vA, mask_sb, id_sb, oT_dram,
                wq_sb, skip_attn=False):
        with tc.tile_pool(name="hsq", bufs=3) as hsp, \
             tc.tile_pool(name="qtb", bufs=2) as qp, \
             tc.tile_pool(name="otb", bufs=1) as op, \
             tc.tile_pool(name="exp", bufs=32) as ep, \
             tc.tile_pool(name="on", bufs=6) as onp:

            def attn_stages(tb):
                """Attention for q-block tb as a list of emission closures.

                Per head: stage1 emits scores+exp (+mask), stage2 emits
                AV/normalize/transpose. Interleaving Q-proj matmuls of the
                NEXT token block between a head's stage1 and stage2 gives
                ScalarE time to produce the exp tiles before the AV matmuls
                need them, so PE never stalls on ACT.
                """
                qTb = attn_stages.qtb[tb]
                oTb = op.tile([128, NH_L * TOKB], bf16, tag="oTb",
                              name=f"oTb{tb}")
                nkc = 4 * tb + 4
                state = {}

                def stage1(h):
                    g = h // GRP_L
                    exps = []
                    for kc in range(nkc):
                        ps_s = ps.tile([128, TOKB], f32, tag="ps")
                        nc.tensor.matmul(
                            ps_s[:],
                            kT[:, g * S + kc * 128: g * S + (kc + 1) * 128],
                            qTb[:, h * TOKB:(h + 1) * TOKB],
                            start=True, stop=True)
                        e = ep.tile([128, TOKB], bf16)
                        nc.scalar.activation(e[:], ps_s[:], af.Exp, scale=SCALE)
                        r = kc - 4 * tb
                        if r >= 0:  # diagonal band: multiplicative 0/1 mask
                            nc.vector.tensor_mul(
                                e[:], e[:], mask_sb[:, r * TOKB:(r + 1) * TOKB])
                        exps.append(e)
                    state[h] = exps

                def stage2(h):
                    g = h // GRP_L
                    exps = state.pop(h)
                    for s2 in range(4):
                        qc = 4 * tb + s2
                        po = ps.tile([128, VSTRIDE], f32, tag="ps")
                        for kc in range(qc + 1):
                            c0 = (g * NQC + kc) * VSTRIDE
                            nc.tensor.matmul(po[:, 0:129],
                                             exps[kc][:, s2 * 128:(s2 + 1) * 128],
                                             vA[:, c0:c0 + 129],
                                             start=(kc == 0), stop=(kc == qc))
                        rcp = onp.tile([128, 1], f32, tag="rcp")
                        nc.vector.reciprocal(rcp[:], po[:, 128:129])
                        on = onp.tile([128, 128], bf16, tag="on")
                        nc.vector.tensor_scalar_mul(on[:], po[:, 0:128], rcp[:])
                        pt = ps.tile([128, 128], bf16, tag="ps")
                        nc.tensor.transpose(pt[:], on[:], id_sb[:])
                        nc.vector.tensor_copy(
                            oTb[:, h * TOKB + s2 * 128: h * TOKB + (s2 + 1) * 128],
                            pt[:])

                def flush():
                    nc.sync.dma_start(
                        oT_dram[:, tb * NH_L * TOKB:(tb + 1) * NH_L * TOKB],
                        oTb[:])

                # one ACT-producer lead slot: s1(0), s1(1), s2(0), s1(2), ...
                stages = [lambda h=0: stage1(0)]
                for h in range(1, NH_L):
                    stages.append(lambda h=h: stage1(h))
                    stages.append(lambda h=h - 1: stage2(h))
                stages.append(lambda: stage2(NH_L - 1))
                stages.append(flush)
                return stages

            attn_stages.qtb = {}

            def emit_q(tb, pending):
                """Q projection for tb; `pending` = attention stages for
                tb-1, drained at intervals between PSUM kc-chunks."""
                tsl = slice(tb * TOKB, (tb + 1) * TOKB)
                hts = []
                for kh in range(2):
                    ht = hsp.tile([128, KHALF * TOKB], bf16, tag="hs",
                                  name=f"hsq{tb}_{kh}")
                    nc.sync.dma_start(
                        ht[:], hs_d[tb * 128:(tb + 1) * 128,
                                    kh * KHALF * TOKB:(kh + 1) * KHALF * TOKB])
                    hts.append(ht)
                qTb = qp.tile([128, NH_L * TOKB], bf16, tag="qTb",
                              name=f"qTb{tb}")
                attn_stages.qtb[tb] = qTb
                # 64 (half, kc) steps; drain one pending stage every 4 steps
                step = 0
                for half in range(2):
                    pqs = [ps.tile([128, TOKB], f32, tag="ps",
                                   name=f"pq{tb}_{half}_{i}") for i in range(4)]
                    for kc in range(NKC):
                        ht = hts[kc // KHALF]
                        hsl = slice((kc % KHALF) * TOKB, (kc % KHALF + 1) * TOKB)
                        for i in range(4):
                            h = half * 4 + i
                            c0 = kc * 1024 + h * 128
                            nc.tensor.matmul(
                                pqs[i][:], wq_sb[:, c0:c0 + 128], ht[:, hsl],
                                start=(kc == 0), stop=(kc == NKC - 1))
                        step += 1
                        if step % 4 == 0 and pending:
                            pending.pop(0)()
                    for i in range(4):
                        h = half * 4 + i
                        _rope(nc, rp, pqs[i], cos_sb, sinn_sb, tsl, qTb,
                              h * TOKB, f32)
                while pending:
                    pending.pop(0)()

            if skip_attn:
                for tb in range(NTB):
                    emit_q(tb, [])
                    qTb = attn_stages.qtb[tb]
                    oTb = op.tile([128, NH_L * TOKB], bf16, tag="oTb",
                                  name=f"oTb{tb}")
                    nc.vector.tensor_copy(oTb[:], qTb[:])
                    nc.sync.dma_start(
                        oT_dram[:, tb * NH_L * TOKB:(tb + 1) * NH_L * TOKB],
                        oTb[:])
                return

            pending = []
            for tb in range(NTB):
                emit_q(tb, pending)
                pending = attn_stages(tb)
            for st in pending:
                st()

    def emit_o(tc, ps, oT_dram):
        with tc.tile_pool(name="sc2", bufs=1) as sc2, \
             tc.tile_pool(name="yrow", bufs=2) as yp:
            wo_sb = sc2.tile([128, NH_L * H], bf16, tag="wo")
            nc.sync.dma_start(wo_sb[:], wo_d[:])
            oTdc = sc2.tile([128, NH_L * S], bf16, tag="oTdc")
            # oTdc[dc] cols t*128 (t = tb*4+s2)  <-  oT_dram col tb*4096+dc*512+s2*128
            for dc in range(NH_L):
                for tb in range(NTB):
                    nc.sync.dma_start(
                        oTdc[:, dc * S + tb * TOKB: dc * S + (tb + 1) * TOKB],
                        oT_dram[:, tb * NH_L * TOKB + dc * TOKB:
                                tb * NH_L * TOKB + (dc + 1) * TOKB])
            for t in range(NQC):
                yr = yp.tile([128, H], f32, tag="yr", name=f"yr{t}")
                for hb in range(H // 512):
                    py = ps.tile([128, 512], f32, tag="ps")
                    for dc in range(NH_L):
                        nc.tensor.matmul(
                            py[:],
                            oTdc[:, dc * S + t * 128: dc * S + (t + 1) * 128],
                            wo_sb[:, dc * H + hb * 512: dc * H + (hb + 1) * 512],
                            start=(dc == 0), stop=(dc == NH_L - 1))
                    nc.scalar.copy(yr[:, hb * 512:(hb + 1) * 512], py[:])
                nc.sync.dma_start(y_d[t * 128:(t + 1) * 128, :], yr[:])

    with tile.TileContext(nc) as tc:
        if reps == 1:
            with ExitStack() as ctx:
                emit(ctx, tc)
        else:
            with tc.For_i(0, reps, 1):
                with ExitStack() as ctx:
                    emit(ctx, tc)
    nc.compile()
    return nc


def get_nc(reps=1):
    if reps not in _NC_CACHE:
        _NC_CACHE[reps] = _build(reps)
    return _NC_CACHE[reps]


def make_in_maps(hidden_states, position_ids, wq, wk, wv, wo):
    hidden_states = np.asarray(hidden_states, dtype=np.float32)
    position_ids = np.asarray(position_ids)
    wq = np.asarray(wq, dtype=np.float32)
    wk = np.asarray(wk, dtype=np.float32)
    wv = np.asarray(wv, dtype=np.float32)
    wo = np.asarray(wo, dtype=np.float32)

    j = np.arange(64, dtype=np.float64)
    invf = 1.0 / (THETA ** (2.0 * j / HD))       # [64]
    kp = np.arange(128)[:, None]
    qf = np.arange(TOKB)[None, :]
    maskB = np.empty((128, 4 * TOKB), dtype=BF16)
    for r in range(4):
        maskB[:, r * TOKB:(r + 1) * TOKB] = (qf >= kp + 128 * r).astype(BF16)
    ident = np.eye(128, dtype=BF16)

    def interleave(wT, n):
        # [H_in, n] fp32 -> [128, (H_in/128)*n] bf16; row p holds all
        # contraction chunks for partition p (direct SBUF image)
        hin = wT.shape[0]
        return np.ascontiguousarray(
            wT.reshape(hin // 128, 128, n).transpose(1, 0, 2).reshape(128, -1)
        ).astype(BF16)

    in_maps = []
    for c in range(N_CORES):
        b, t = divmod(c, TP)
        pos = position_ids[b].astype(np.float64)     # [S]
        freqs = pos[:, None] * invf[None, :]         # [S, 64]
        cos64 = np.cos(freqs).astype(np.float16).T   # [64, S]
        sin64 = np.sin(freqs).astype(np.float16).T
        cosT = np.ascontiguousarray(np.concatenate([cos64, cos64], axis=0))
        sinN = np.ascontiguousarray(np.concatenate([-sin64, sin64], axis=0))

        hsT = hidden_states[b].T                     # [H, S] fp32
        # rows (tb*128 + p), cols (kc*512 + c)
        hs_i = np.ascontiguousarray(
            hsT.reshape(NKC, 128, NTB, TOKB).transpose(2, 1, 0, 3)
            .reshape(NTB * 128, NKC * TOKB)).astype(BF16)

        in_maps.append({
            "hs": hs_i,
            "wq": interleave(
                np.ascontiguousarray(wq[t * NH_L * HD:(t + 1) * NH_L * HD, :].T),
                NH_L * HD),
            "wk": interleave(
                np.ascontiguousarray(wk[t * NKV_L * HD:(t + 1) * NKV_L * HD, :].T),
                NKV_L * HD),
            "wv": interleave(
                np.ascontiguousarray(wv[t * NKV_L * HD:(t + 1) * NKV_L * HD, :].T),
                NKV_L * HD),
            "wo": interleave(
                np.ascontiguousarray(wo[:, t * NH_L * HD:(t + 1) * NH_L * HD].T), H),
            "cosT": cosT,
            "sinN": sinN,
            "maskB": maskB,
            "ident": ident,
        })
    return in_maps


def gather_out(results):
    """results: list of 8 dicts with 'y' [S, H] fp32 -> [B, S, H] fp32."""
    out = np.zeros((B, S, H), dtype=np.float32)
    for c in range(N_CORES):
        b = c // TP
        out[b] += results[c]["y"]
    return out


def kernel(**inputs):
    from concourse.bass_utils import run_bass_kernel_spmd

    nc = get_nc(reps=1)
    in_maps = make_in_maps(**inputs)
    res = run_bass_kernel_spmd(nc, in_maps, core_ids=list(range(N_CORES)))
    return gather_out(res.results)



# revision 2
# speedup vs baseline: 2.3475x; 2.3475x over previous
"""LlamaAttention (B=2, S=2048, H=4096, 32 q heads / 8 kv heads, RoPE, causal)
on 8 Trainium2 NeuronCores.

Sharding: data-parallel over batch (2) x tensor-parallel over heads (4).
Core c = b*4 + t handles batch b with q heads 8t..8t+7 and kv heads 2t..2t+1.
Each core computes a partial output y_c = attn_out_local @ wo_local^T
([S, H], bf16); the host sums the 4 TP partials per batch in fp32.

All matmuls bf16 inputs / fp32 PSUM accumulation. All weight/activation
DRAM tensors are pre-interleaved on the host so each DMA is a direct image
of its SBUF destination.

v4 structure (vs v3: no DRAM oT spill, kc-granular attention pipeline,
disciplined 8-bank PSUM budget, KV-first DMA order, bf16 y):
  KV phase : kT[d, s] per kv head (+ fused RoPE); v in natural [s, d] layout
             with a ones column (vAug) so the softmax denominator falls out
             of the AV matmul for free.  PSUM: pk x2 + pv x4 = 6 banks.
  QA phase : per 512-token block tb: Q-proj as 4 passes of 2 heads (PSUM
             pq x3 rotating) + RoPE -> qTb; then causal attention for all
             heads: per head, scores for all kc chunks (sT[k,q] =
             kT_chunk.T @ qTb, PSUM sp x2) -> exp on ACT (bf16, no
             max-subtraction; scores are O(1)) -> multiplicative 0/1
             diagonal masks -> per s2: AV with vAug (PSUM po x2) ->
             normalize by the ones column -> PE transpose (pt x1) ->
             oT_sb[d, s] in SBUF.  PSUM: 3+2+2+1 = 8 banks.
  O phase  : y[t, :] = sum_h oT_h[:,t].T @ wo_h, wo streamed in 8
             hb-slices so compute starts ~3us after QA; y stored bf16.
"""
import sys

sys.path.insert(0, "/opt/trn_rl_repo")

import numpy as np
import ml_dtypes

BF16 = ml_dtypes.bfloat16

B, S, H = 2, 2048, 4096
NH, NKV, HD = 32, 8, 128
THETA = 10000.0
SCALE = 1.0 / float(np.sqrt(HD))

N_CORES = 8
TP = 4
NH_L = NH // TP        # 8 local q heads
NKV_L = NKV // TP      # 2 local kv heads
GRP_L = NH_L // NKV_L  # 4 q heads per local kv head
TOKB = 512
NKC = H // 128         # 32 contraction chunks
NTB = S // TOKB        # 4 token blocks
NQC = S // 128         # 16 token chunks
VSTRIDE = 132          # per-chunk stride in vAug (129 used, pad for alignment)
KHALF = NKC // 2

_NC_CACHE = {}


def _rope(nc, rp, psum, cos_sb, sinn_sb, tsl, outT, col0, f32):
    """RoPE on a [128(d), TOKB] fp32 PSUM block; writes bf16 to outT[:, col0:+TOKB].

    out[0:64]   = p[0:64]*cos - p[64:128]*sin
    out[64:128] = p[64:128]*cos + p[0:64]*sin
    (cos rows duplicated; sinn rows 0:64 pre-negated on host; fp16 tables.)
    """
    tcos = rp.tile([128, TOKB], f32, tag="tcos")
    nc.vector.tensor_mul(tcos[:], psum[:], cos_sb[:, tsl])
    trs = rp.tile([128, TOKB], f32, tag="trs")
    nc.vector.tensor_mul(trs[0:64, :], psum[64:128, :], sinn_sb[0:64, tsl])
    nc.vector.tensor_mul(trs[64:128, :], psum[0:64, :], sinn_sb[64:128, tsl])
    nc.vector.tensor_add(outT[:, col0: col0 + TOKB], tcos[:], trs[:])


def _build(reps=1, phases="full"):
    import concourse.mybir as mybir
    import concourse.tile as tile
    from concourse import bacc
    from contextlib import ExitStack

    dt = mybir.dt
    f32, bf16, f16 = dt.float32, dt.bfloat16, dt.float16
    af = mybir.ActivationFunctionType

    nc = bacc.Bacc("TRN2", target_bir_lowering=False, debug=False,
                   enable_asserts=True, num_devices=N_CORES)
    # all pre-interleaved on host: DMA source rows == SBUF partition images
    hs_d = nc.dram_tensor("hs", [NTB * 128, NKC * TOKB], bf16, kind="ExternalInput").ap()
    wq_d = nc.dram_tensor("wq", [128, NKC * NH_L * 128], bf16, kind="ExternalInput").ap()
    wk_d = nc.dram_tensor("wk", [128, NKC * NKV_L * 128], bf16, kind="ExternalInput").ap()
    wv_d = nc.dram_tensor("wv", [128, NKC * NKV_L * 128], bf16, kind="ExternalInput").ap()
    wo_d = nc.dram_tensor("wo", [128, NH_L * H], bf16, kind="ExternalInput").ap()
    cos_d = nc.dram_tensor("cosT", [128, S], f16, kind="ExternalInput").ap()
    sin_d = nc.dram_tensor("sinN", [128, S], f16, kind="ExternalInput").ap()
    msk_d = nc.dram_tensor("maskB", [128, 4 * TOKB], bf16, kind="ExternalInput").ap()
    id_d = nc.dram_tensor("ident", [128, 128], bf16, kind="ExternalInput").ap()
    y_d = nc.dram_tensor("y", [S, H], bf16, kind="ExternalOutput").ap()

    def emit_kv(tc, rp, cos_sb, sinn_sb, kT, vA):
        with tc.tile_pool(name="wkv", bufs=1) as wkvp, \
             tc.tile_pool(name="hskv", bufs=3) as hsp, \
             tc.tile_pool(name="pk", bufs=2, space="PSUM") as pkp, \
             tc.tile_pool(name="pv", bufs=4, space="PSUM") as pvp:
            wk_sb = wkvp.tile([128, NKC * NKV_L * 128], bf16, tag="wk")
            nc.sync.dma_start(wk_sb[:], wk_d[:])
            wv_sb = wkvp.tile([128, NKC * NKV_L * 128], bf16, tag="wv")
            nc.sync.dma_start(wv_sb[:], wv_d[:])
            for tb in range(NTB):
                tsl = slice(tb * TOKB, (tb + 1) * TOKB)
                pks = [pkp.tile([128, TOKB], f32, tag="pk", name=f"pk{tb}_{i}")
                       for i in range(NKV_L)]
                pvs = [pvp.tile([128, 256], f32, tag="pv", name=f"pv{tb}_{i}")
                       for i in range(4)]
                for kh in range(2):
                    ht = hsp.tile([128, KHALF * TOKB], bf16, tag="hs",
                                  name=f"hskv{tb}_{kh}")
                    nc.sync.dma_start(
                        ht[:], hs_d[tb * 128:(tb + 1) * 128,
                                    kh * KHALF * TOKB:(kh + 1) * KHALF * TOKB])
                    for k2 in range(KHALF):
                        kc = kh * KHALF + k2
                        hsl = slice(k2 * TOKB, (k2 + 1) * TOKB)
                        for g in range(NKV_L):
                            c0 = kc * 256 + g * 128
                            nc.tensor.matmul(
                                pks[g][:], wk_sb[:, c0:c0 + 128], ht[:, hsl],
                                start=(kc == 0), stop=(kc == NKC - 1))
                        for s in range(4):
                            nc.tensor.matmul(
                                pvs[s][:],
                                ht[:, k2 * TOKB + s * 128: k2 * TOKB + (s + 1) * 128],
                                wv_sb[:, kc * 256:(kc + 1) * 256],
                                start=(kc == 0), stop=(kc == NKC - 1))
                for g in range(NKV_L):
                    _rope(nc, rp, pks[g], cos_sb, sinn_sb, tsl, kT, g * S + tb * TOKB, f32)
                for s in range(4):
                    qc = tb * 4 + s
                    for g in range(NKV_L):
                        c0 = (g * NQC + qc) * VSTRIDE
                        nc.vector.tensor_copy(
                            vA[:, c0:c0 + 128],
                            pvs[s][:, g * 128:(g + 1) * 128])

    def emit_qa(tc, rp, cos_sb, sinn_sb, kT, vA, mask_sb, id_sb, oT,
                skip_attn=False):
        with tc.tile_pool(name="wq", bufs=1) as wqp, \
             tc.tile_pool(name="hsq", bufs=2) as hsp, \
             tc.tile_pool(name="qtb", bufs=2) as qp, \
             tc.tile_pool(name="exp", bufs=20) as ep, \
             tc.tile_pool(name="on", bufs=4) as onp, \
             tc.tile_pool(name="pq", bufs=3, space="PSUM") as pqp, \
             tc.tile_pool(name="sp", bufs=2, space="PSUM") as spp, \
             tc.tile_pool(name="po", bufs=2, space="PSUM") as pop, \
             tc.tile_pool(name="pt", bufs=1, space="PSUM") as ptp:
            wq_sb = wqp.tile([128, NKC * NH_L * 128], bf16, tag="wq")
            nc.sync.dma_start(wq_sb[:], wq_d[:])

            def emit_q(tb):
                """Q projection for tb: 4 passes of 2 heads over resident hs."""
                tsl = slice(tb * TOKB, (tb + 1) * TOKB)
                hts = []
                for kh in range(2):
                    ht = hsp.tile([128, KHALF * TOKB], bf16, tag="hs",
                                  name=f"hsq{tb}_{kh}")
                    nc.sync.dma_start(
                        ht[:], hs_d[tb * 128:(tb + 1) * 128,
                                    kh * KHALF * TOKB:(kh + 1) * KHALF * TOKB])
                    hts.append(ht)
                qTb = qp.tile([128, NH_L * TOKB], bf16, tag="qTb",
                              name=f"qTb{tb}")
                for p in range(4):
                    pqs = [pqp.tile([128, TOKB], f32, tag="pq",
                                    name=f"pq{tb}_{p}_{i}") for i in range(2)]
                    for kc in range(NKC):
                        ht = hts[kc // KHALF]
                        hsl = slice((kc % KHALF) * TOKB, (kc % KHALF + 1) * TOKB)
                        for i in range(2):
                            h = 2 * p + i
                            c0 = kc * 1024 + h * 128
                            nc.tensor.matmul(
                                pqs[i][:], wq_sb[:, c0:c0 + 128], ht[:, hsl],
                                start=(kc == 0), stop=(kc == NKC - 1))
                    for i in range(2):
                        _rope(nc, rp, pqs[i], cos_sb, sinn_sb, tsl, qTb,
                              (2 * p + i) * TOKB, f32)
                return qTb

            def emit_attn(tb, qTb):
                """Causal attention for q-block tb, kc-granular pipeline."""
                nkc = 4 * tb + 4
                for h in range(NH_L):
                    g = h // GRP_L
                    exps = []
                    for kc in range(nkc):
                        sps = spp.tile([128, TOKB], f32, tag="sp")
                        nc.tensor.matmul(
                            sps[:],
                            kT[:, g * S + kc * 128: g * S + (kc + 1) * 128],
                            qTb[:, h * TOKB:(h + 1) * TOKB],
                            start=True, stop=True)
                        e = ep.tile([128, TOKB], bf16, tag="e")
                        nc.scalar.activation(e[:], sps[:], af.Exp, scale=SCALE)
                        r = kc - 4 * tb
                        if r >= 0:  # diagonal band: multiplicative 0/1 mask
                            nc.vector.tensor_mul(
                                e[:], e[:], mask_sb[:, r * TOKB:(r + 1) * TOKB])
                        exps.append(e)
                    for s2 in range(4):
                        qc = 4 * tb + s2
                        po = pop.tile([128, VSTRIDE], f32, tag="po")
                        for kc in range(qc + 1):
                            c0 = (g * NQC + kc) * VSTRIDE
                            nc.tensor.matmul(po[:, 0:129],
                                             exps[kc][:, s2 * 128:(s2 + 1) * 128],
                                             vA[:, c0:c0 + 129],
                                             start=(kc == 0), stop=(kc == qc))
                        rcp = onp.tile([128, 1], f32, tag="rcp")
                        nc.vector.reciprocal(rcp[:], po[:, 128:129])
                        on = onp.tile([128, 128], bf16, tag="on")
                        nc.vector.tensor_scalar_mul(on[:], po[:, 0:128], rcp[:])
                        pt = ptp.tile([128, 128], bf16, tag="pt")
                        nc.tensor.transpose(pt[:], on[:], id_sb[:])
                        nc.vector.tensor_copy(
                            oT[:, h * S + tb * TOKB + s2 * 128:
                               h * S + tb * TOKB + (s2 + 1) * 128],
                            pt[:])

            for tb in range(NTB):
                qTb = emit_q(tb)
                if skip_attn:
                    nc.vector.tensor_copy(
                        oT[:, 0 * S + tb * TOKB: 0 * S + (tb + 1) * TOKB],
                        qTb[:, 0:TOKB])
                else:
                    emit_attn(tb, qTb)

    def emit_o(tc, oT):
        with tc.tile_pool(name="wo", bufs=1) as wop, \
             tc.tile_pool(name="yrow", bufs=2) as yp, \
             tc.tile_pool(name="py", bufs=2, space="PSUM") as pyp:
            wo_sb = wop.tile([128, NH_L * H], bf16, tag="wo")
            # hb-sliced loads so t=0/hb=0 compute starts after ~1/8 of wo
            for hb in range(H // 512):
                for h in range(NH_L):
                    c0 = h * H + hb * 512
                    nc.sync.dma_start(wo_sb[:, c0:c0 + 512],
                                      wo_d[:, c0:c0 + 512])
            for t in range(NQC):
                yr = yp.tile([128, H], bf16, tag="yr", name=f"yr{t}")
                for hb in range(H // 512):
                    py = pyp.tile([128, 512], f32, tag="py")
                    for h in range(NH_L):
                        nc.tensor.matmul(
                            py[:],
                            oT[:, h * S + t * 128: h * S + (t + 1) * 128],
                            wo_sb[:, h * H + hb * 512: h * H + (hb + 1) * 512],
                            start=(h == 0), stop=(h == NH_L - 1))
                    nc.scalar.copy(yr[:, hb * 512:(hb + 1) * 512], py[:])
                nc.sync.dma_start(y_d[t * 128:(t + 1) * 128, :], yr[:])

    def emit(ctx, tc):
        persist = ctx.enter_context(tc.tile_pool(name="persist", bufs=1))
        csp = ctx.enter_context(tc.tile_pool(name="cs", bufs=1))
        rp = ctx.enter_context(tc.tile_pool(name="rope", bufs=2))

        # KV-phase consumables first so PE starts ASAP; wq/wo stream later.
        mask_sb = persist.tile([128, 4 * TOKB], bf16, tag="mask")
        nc.sync.dma_start(mask_sb[:], msk_d[:])
        id_sb = persist.tile([128, 128], bf16, tag="ident")
        nc.sync.dma_start(id_sb[:], id_d[:])
        cos_sb = csp.tile([128, S], f16, tag="cos")
        nc.sync.dma_start(cos_sb[:], cos_d[:])
        sinn_sb = csp.tile([128, S], f16, tag="sinn")
        nc.sync.dma_start(sinn_sb[:], sin_d[:])
        kT = persist.tile([128, NKV_L * S], bf16, tag="kT")
        vA = persist.tile([128, NKV_L * NQC * VSTRIDE], bf16, tag="vA")
        nc.gpsimd.memset(vA[:], 1.0)
        oT = persist.tile([128, NH_L * S], bf16, tag="oT")

        if phases in ("full", "kv", "kvq", "kvqa"):
            emit_kv(tc, rp, cos_sb, sinn_sb, kT, vA)
        if phases == "qa":
            nc.vector.memset(kT[:], 0.01)
        if phases in ("full", "qa", "kvqa", "kvq"):
            emit_qa(tc, rp, cos_sb, sinn_sb, kT, vA, mask_sb, id_sb, oT,
                    skip_attn=(phases == "kvq"))
        if phases in ("full", "o"):
            if phases == "o":
                nc.vector.memset(oT[:], 0.01)
            emit_o(tc, oT)

    with tile.TileContext(nc) as tc:
        if reps == 1:
            with ExitStack() as ctx:
                emit(ctx, tc)
        else:
            with tc.For_i(0, reps, 1):
                with ExitStack() as ctx:
                    emit(ctx, tc)
    nc.compile()
    return nc


def get_nc(reps=1):
    if reps not in _NC_CACHE:
        _NC_CACHE[reps] = _build(reps)
    return _NC_CACHE[reps]


def make_in_maps(hidden_states, position_ids, wq, wk, wv, wo):
    hidden_states = np.asarray(hidden_states, dtype=np.float32)
    position_ids = np.asarray(position_ids)
    wq = np.asarray(wq, dtype=np.float32)
    wk = np.asarray(wk, dtype=np.float32)
    wv = np.asarray(wv, dtype=np.float32)
    wo = np.asarray(wo, dtype=np.float32)

    j = np.arange(64, dtype=np.float64)
    invf = 1.0 / (THETA ** (2.0 * j / HD))       # [64]
    kp = np.arange(128)[:, None]
    qf = np.arange(TOKB)[None, :]
    maskB = np.empty((128, 4 * TOKB), dtype=BF16)
    for r in range(4):
        maskB[:, r * TOKB:(r + 1) * TOKB] = (qf >= kp + 128 * r).astype(BF16)
    ident = np.eye(128, dtype=BF16)

    def interleave(wT, n):
        # [H_in, n] fp32 -> [128, (H_in/128)*n] bf16; row p holds all
        # contraction chunks for partition p (direct SBUF image)
        hin = wT.shape[0]
        return np.ascontiguousarray(
            wT.reshape(hin // 128, 128, n).transpose(1, 0, 2).reshape(128, -1)
        ).astype(BF16)

    in_maps = []
    for c in range(N_CORES):
        b, t = divmod(c, TP)
        pos = position_ids[b].astype(np.float64)     # [S]
        freqs = pos[:, None] * invf[None, :]         # [S, 64]
        cos64 = np.cos(freqs).astype(np.float16).T   # [64, S]
        sin64 = np.sin(freqs).astype(np.float16).T
        cosT = np.ascontiguousarray(np.concatenate([cos64, cos64], axis=0))
        sinN = np.ascontiguousarray(np.concatenate([-sin64, sin64], axis=0))

        hsT = hidden_states[b].T                     # [H, S] fp32
        # rows (tb*128 + p), cols (kc*512 + c)
        hs_i = np.ascontiguousarray(
            hsT.reshape(NKC, 128, NTB, TOKB).transpose(2, 1, 0, 3)
            .reshape(NTB * 128, NKC * TOKB)).astype(BF16)

        in_maps.append({
            "hs": hs_i,
            "wq": interleave(
                np.ascontiguousarray(wq[t * NH_L * HD:(t + 1) * NH_L * HD, :].T),
                NH_L * HD),
            "wk": interleave(
                np.ascontiguousarray(wk[t * NKV_L * HD:(t + 1) * NKV_L * HD, :].T),
                NKV_L * HD),
            "wv": interleave(
                np.ascontiguousarray(wv[t * NKV_L * HD:(t + 1) * NKV_L * HD, :].T),
                NKV_L * HD),
            "wo": interleave(
                np.ascontiguousarray(wo[:, t * NH_L * HD:(t + 1) * NH_L * HD].T), H),
            "cosT": cosT,
            "sinN": sinN,
            "maskB": maskB,
            "ident": ident,
        })
    return in_maps


def gather_out(results):
    """results: list of 8 dicts with 'y' [S, H] bf16 -> [B, S, H] fp32."""
    out = np.zeros((B, S, H), dtype=np.float32)
    for c in range(N_CORES):
        b = c // TP
        out[b] += results[c]["y"].astype(np.float32)
    return out


def kernel(**inputs):
    from concourse.bass_utils import run_bass_kernel_spmd

    nc = get_nc(reps=1)
    in_maps = make_in_maps(**inputs)
    res = run_bass_kernel_spmd(nc, in_maps, core_ids=list(range(N_CORES)))
    return gather_out(res.results)


# revision 19
# speedup vs baseline: 4.3321x; 1.8455x over previous
"""LlamaAttention (B=2, S=2048, H=4096, 32 q heads / 8 kv heads, RoPE, causal)
on 8 Trainium2 NeuronCores.

Sharding: data-parallel over batch (2) x tensor-parallel over heads (4).
Core c = b*4 + t handles batch b with q heads 8t..8t+7 and kv heads 2t..2t+1.
Each core computes a partial output y_c = attn_out_local @ wo_local^T
([S, H], bf16); the host sums the 4 TP partials per batch in fp32.

All matmuls bf16 inputs / fp32 PSUM accumulation. All weight/activation
DRAM tensors are pre-interleaved on the host so each DMA is a direct image
of its SBUF destination.

v4 structure (vs v3: no DRAM oT spill, kc-granular attention pipeline,
disciplined 8-bank PSUM budget, KV-first DMA order, bf16 y):
  KV phase : kT[d, s] per kv head (+ fused RoPE); v in natural [s, d] layout
             with a ones column (vAug) so the softmax denominator falls out
             of the AV matmul for free.  PSUM: pk x2 + pv x4 = 6 banks.
  QA phase : per 512-token block tb: Q-proj as 4 passes of 2 heads (PSUM
             pq x3 rotating) + RoPE -> qTb; then causal attention for all
             heads: per head, scores for all kc chunks (sT[k,q] =
             kT_chunk.T @ qTb, PSUM sp x2) -> exp on ACT (bf16, no
             max-subtraction; scores are O(1)) -> multiplicative 0/1
             diagonal masks -> per s2: AV with vAug (PSUM po x2) ->
             normalize by the ones column -> PE transpose (pt x1) ->
             oT_sb[d, s] in SBUF.  PSUM: 3+2+2+1 = 8 banks.
  O phase  : y[t, :] = sum_h oT_h[:,t].T @ wo_h, wo streamed in 8
             hb-slices so compute starts ~3us after QA; y stored bf16.
"""
import sys

sys.path.insert(0, "/opt/trn_rl_repo")

import numpy as np
import ml_dtypes

BF16 = ml_dtypes.bfloat16

B, S, H = 2, 2048, 4096
NH, NKV, HD = 32, 8, 128
THETA = 10000.0
SCALE = 1.0 / float(np.sqrt(HD))

N_CORES = 8
TP = 4
NH_L = NH // TP        # 8 local q heads
NKV_L = NKV // TP      # 2 local kv heads
GRP_L = NH_L // NKV_L  # 4 q heads per local kv head
TOKB = 512
NKC = H // 128         # 32 contraction chunks
NTB = S // TOKB        # 4 token blocks
NQC = S // 128         # 16 token chunks
VSTRIDE = 132          # per-chunk stride in vAug (129 used, pad for alignment)
KHALF = NKC // 2

_NC_CACHE = {}


def _rope(nc, rp, psum, cos_sb, sinn_sb, tsl, outT, col0, f32):
    """RoPE on a [128(d), TOKB] fp32 PSUM block; writes bf16 to outT[:, col0:+TOKB].

    out[0:64]   = p[0:64]*cos - p[64:128]*sin
    out[64:128] = p[64:128]*cos + p[0:64]*sin
    (cos rows duplicated; sinn rows 0:64 pre-negated on host; fp16 tables.)
    """
    tcos = rp.tile([128, TOKB], f32, tag="tcos")
    nc.vector.tensor_mul(tcos[:], psum[:], cos_sb[:, tsl])
    trs = rp.tile([128, TOKB], f32, tag="trs")
    nc.vector.tensor_mul(trs[0:64, :], psum[64:128, :], sinn_sb[0:64, tsl])
    nc.vector.tensor_mul(trs[64:128, :], psum[0:64, :], sinn_sb[64:128, tsl])
    nc.vector.tensor_add(outT[:, col0: col0 + TOKB], tcos[:], trs[:])


def _build(reps=1, phases="full"):
    import concourse.mybir as mybir
    import concourse.tile as tile
    from concourse import bacc
    from contextlib import ExitStack

    dt = mybir.dt
    f32, bf16, f16 = dt.float32, dt.bfloat16, dt.float16
    af = mybir.ActivationFunctionType

    nc = bacc.Bacc("TRN2", target_bir_lowering=False, debug=False,
                   enable_asserts=True, num_devices=N_CORES)
    # all pre-interleaved on host: DMA source rows == SBUF partition images
    hs_d = nc.dram_tensor("hs", [NTB * 128, NKC * TOKB], bf16, kind="ExternalInput").ap()
    wq_d = nc.dram_tensor("wq", [128, NKC * NH_L * 128], bf16, kind="ExternalInput").ap()
    wk_d = nc.dram_tensor("wk", [128, NKC * NKV_L * 128], bf16, kind="ExternalInput").ap()
    wv_d = nc.dram_tensor("wv", [128, NKC * NKV_L * 128], bf16, kind="ExternalInput").ap()
    wo_d = nc.dram_tensor("wo", [128, NH_L * H], bf16, kind="ExternalInput").ap()
    cos_d = nc.dram_tensor("cosT", [128, S], f16, kind="ExternalInput").ap()
    sin_d = nc.dram_tensor("sinN", [128, S], f16, kind="ExternalInput").ap()
    msk_d = nc.dram_tensor("maskB", [128, 128], bf16, kind="ExternalInput").ap()
    id_d = nc.dram_tensor("ident", [128, 128], bf16, kind="ExternalInput").ap()
    y_d = nc.dram_tensor("y", [S, H], bf16, kind="ExternalOutput").ap()

    def emit_kv(tc, rp, cos_sb, sinn_sb, mask_sb, id_sb, kT, vA):
        # PSUM: pk x2 + pv x2 = 4 banks, leaving 4 free so the QA phase's
        # pq pool (3 banks) can start before the KV tail fully drains.
        with tc.tile_pool(name="wkv", bufs=1) as wkvp, \
             tc.tile_pool(name="hskv", bufs=4) as hsp, \
             tc.tile_pool(name="pk", bufs=3, space="PSUM") as pkp, \
             tc.tile_pool(name="pv", bufs=2, space="PSUM") as pvp:
            # First hs tile + wv on the Pool queue, wk on SP: both engines
            # stream concurrently so the first matmul starts ~7us in.
            # (hskv stays at 4 bufs = 64KB/p so SBUF has room for wq to
            # stream in DURING the KV phase -- at 6 bufs the allocator
            # delays the wq DMA to the end of KV.)
            ht0 = hsp.tile([128, KHALF * TOKB], bf16, tag="hs", name="hskv0_0")
            nc.gpsimd.dma_start(ht0[:], hs_d[0:128, 0:KHALF * TOKB])
            wk_sb = wkvp.tile([128, NKC * NKV_L * 128], bf16, tag="wk")
            nc.sync.dma_start(wk_sb[:], wk_d[:])
            wv_sb = wkvp.tile([128, NKC * NKV_L * 128], bf16, tag="wv")
            nc.gpsimd.dma_start(wv_sb[:], wv_d[:])
            nc.sync.dma_start(cos_sb[:], cos_d[:])
            nc.sync.dma_start(sinn_sb[:], sin_d[:])
            nc.sync.dma_start(mask_sb[:], msk_d[:])
            nc.sync.dma_start(id_sb[:], id_d[:])
            # prefetch remaining hs tiles (slot-paced at bufs=6; emitted
            # after the small tables so they never block them)
            hts = {(0, 0): ht0}
            for tb in range(NTB):
                for kh in range(2):
                    if (tb, kh) not in hts:
                        ht = hsp.tile([128, KHALF * TOKB], bf16, tag="hs",
                                      name=f"hskv{tb}_{kh}")
                        nc.sync.dma_start(
                            ht[:], hs_d[tb * 128:(tb + 1) * 128,
                                        kh * KHALF * TOKB:(kh + 1) * KHALF * TOKB])
                        hts[(tb, kh)] = ht
            for tb in range(NTB):
                tsl = slice(tb * TOKB, (tb + 1) * TOKB)
                pks = [pkp.tile([128, TOKB], f32, tag="pk", name=f"pk{tb}_{i}")
                       for i in range(NKV_L)]
                for sp in range(2):  # two V sub-passes of 2 s-slices each
                    pvs = [pvp.tile([128, 256], f32, tag="pv",
                                    name=f"pv{tb}_{sp}_{i}") for i in range(2)]
                    for kh in range(2):
                        ht = hts[(tb, kh)]
                        for k2 in range(KHALF):
                            kc = kh * KHALF + k2
                            hsl = slice(k2 * TOKB, (k2 + 1) * TOKB)
                            if sp == 0:  # K rides the first sub-pass
                                for g in range(NKV_L):
                                    c0 = kc * 256 + g * 128
                                    nc.tensor.matmul(
                                        pks[g][:], wk_sb[:, c0:c0 + 128],
                                        ht[:, hsl],
                                        start=(kc == 0), stop=(kc == NKC - 1))
                            for i in range(2):
                                s = 2 * sp + i
                                nc.tensor.matmul(
                                    pvs[i][:],
                                    ht[:, k2 * TOKB + s * 128:
                                       k2 * TOKB + (s + 1) * 128],
                                    wv_sb[:, kc * 256:(kc + 1) * 256],
                                    start=(kc == 0), stop=(kc == NKC - 1))
                    for i in range(2):
                        s = 2 * sp + i
                        qc = tb * 4 + s
                        for g in range(NKV_L):
                            c0 = (g * NQC + qc) * VSTRIDE
                            nc.vector.tensor_copy(
                                vA[:, c0:c0 + 128],
                                pvs[i][:, g * 128:(g + 1) * 128])
                for g in range(NKV_L):
                    _rope(nc, rp, pks[g], cos_sb, sinn_sb, tsl, kT,
                          g * S + tb * TOKB, f32)

    def emit_qao(tc, rp, cos_sb, sinn_sb, kT, vA, mask_sb, id_sb, oT,
                 skip_attn=False, skip_o=False):
        with tc.tile_pool(name="qtb", bufs=2) as qp, \
             tc.tile_pool(name="exp", bufs=20) as ep, \
             tc.tile_pool(name="on", bufs=4) as onp, \
             tc.tile_pool(name="sp", bufs=2, space="PSUM") as spp, \
             tc.tile_pool(name="po", bufs=2, space="PSUM") as pop, \
             tc.tile_pool(name="pt", bufs=1, space="PSUM") as ptp:

            def attn_steps(tb, qTb):
                """Attention micro-steps for q-block tb: per head, one step
                per score chunk (matmul+exp+mask), then one per AV group.
                Drained interleaved under Q-proj(tb+1) / O-proj so the PE
                keeps dense work while ACT computes the exps."""
                nkc = 4 * tb + 4
                steps = []
                for h in range(NH_L):
                    g = h // GRP_L
                    exps = []
                    for kc in range(nkc):
                        def sstep(h=h, g=g, exps=exps, kc=kc, tb=tb, qTb=qTb):
                            # diagonal chunk r: q-columns < 128r are above the
                            # causal diagonal; skip them (AV only reads
                            # s2 >= r slices, so the stale region is unread).
                            r = max(0, kc - 4 * tb)
                            q0 = 128 * r
                            sps = spp.tile([128, TOKB], f32, tag="sp")
                            nc.tensor.matmul(
                                sps[:, 0:TOKB - q0],
                                kT[:, g * S + kc * 128: g * S + (kc + 1) * 128],
                                qTb[:, h * TOKB + q0:(h + 1) * TOKB],
                                start=True, stop=True)
                            e = ep.tile([128, TOKB], bf16, tag="e")
                            nc.scalar.activation(e[:, q0:TOKB],
                                                 sps[:, 0:TOKB - q0],
                                                 af.Exp, scale=SCALE)
                            if kc - 4 * tb >= 0:
                                # triangle mask on the leading 128-block
                                nc.vector.tensor_mul(
                                    e[:, q0:q0 + 128], e[:, q0:q0 + 128],
                                    mask_sb[:, 0:128])
                            exps.append(e)
                        steps.append(sstep)
                    for s2 in range(4):
                        def avstep(h=h, g=g, exps=exps, s2=s2, tb=tb):
                            qc = 4 * tb + s2
                            po = pop.tile([128, VSTRIDE], f32, tag="po")
                            for kc in range(qc + 1):
                                c0 = (g * NQC + kc) * VSTRIDE
                                nc.tensor.matmul(
                                    po[:, 0:129],
                                    exps[kc][:, s2 * 128:(s2 + 1) * 128],
                                    vA[:, c0:c0 + 129],
                                    start=(kc == 0), stop=(kc == qc))
                            rcp = onp.tile([128, 1], f32, tag="rcp")
                            nc.vector.reciprocal(rcp[:], po[:, 128:129])
                            on = onp.tile([128, 128], bf16, tag="on")
                            nc.vector.tensor_scalar_mul(on[:], po[:, 0:128], rcp[:])
                            pt = ptp.tile([128, 128], bf16, tag="pt")
                            nc.tensor.transpose(pt[:], on[:], id_sb[:])
                            nc.vector.tensor_copy(
                                oT[:, h * S + tb * TOKB + s2 * 128:
                                   h * S + tb * TOKB + (s2 + 1) * 128],
                                pt[:])
                        steps.append(avstep)
                return steps

            with tc.tile_pool(name="wq", bufs=1) as wqp, \
                 tc.tile_pool(name="hsq", bufs=2) as hsp, \
                 tc.tile_pool(name="pq", bufs=3, space="PSUM") as pqp:
                # wq in two 32KB/p halves (heads 0-3 / 4-7, host-interleaved
                # contiguously) on the Pool-engine DGE queue: the first half
                # fits in SBUF alongside the KV working set and streams in
                # at ~15us; a single 64KB tile would wait for KV to drain.
                wq_sbs = []
                for w2 in range(2):
                    wq_sb = wqp.tile([128, NKC * 4 * 128], bf16,
                                     tag=f"wq{w2}")
                    nc.gpsimd.dma_start(
                        wq_sb[:],
                        wq_d[:, w2 * NKC * 512:(w2 + 1) * NKC * 512])
                    wq_sbs.append(wq_sb)

                def emit_q(tb, pending):
                    """Q projection for tb (4 passes of 2 heads over resident
                    hs), draining attention steps of tb-1 between kc steps."""
                    tsl = slice(tb * TOKB, (tb + 1) * TOKB)
                    hts = []
                    for kh in range(2):
                        ht = hsp.tile([128, KHALF * TOKB], bf16, tag="hs",
                                      name=f"hsq{tb}_{kh}")
                        nc.gpsimd.dma_start(
                            ht[:], hs_d[tb * 128:(tb + 1) * 128,
                                        kh * KHALF * TOKB:(kh + 1) * KHALF * TOKB])
                        hts.append(ht)
                    qTb = qp.tile([128, NH_L * TOKB], bf16, tag="qTb",
                                  name=f"qTb{tb}")
                    per = len(pending) / 128.0
                    acc = 0.0
                    for p in range(4):
                        pqs = [pqp.tile([128, TOKB], f32, tag="pq",
                                        name=f"pq{tb}_{p}_{i}") for i in range(2)]
                        wq_sb = wq_sbs[p // 2]
                        for kc in range(NKC):
                            ht = hts[kc // KHALF]
                            hsl = slice((kc % KHALF) * TOKB,
                                        (kc % KHALF + 1) * TOKB)
                            for i in range(2):
                                hh = (2 * p + i) % 4
                                c0 = kc * 512 + hh * 128
                                nc.tensor.matmul(
                                    pqs[i][:], wq_sb[:, c0:c0 + 128], ht[:, hsl],
                                    start=(kc == 0), stop=(kc == NKC - 1))
                            acc += per
                            while acc >= 1.0 and pending:
                                pending.pop(0)()
                                acc -= 1.0
                        for i in range(2):
                            _rope(nc, rp, pqs[i], cos_sb, sinn_sb, tsl, qTb,
                                  (2 * p + i) * TOKB, f32)
                    while pending:
                        pending.pop(0)()
                    return qTb

                pending = []
                for tb in range(NTB):
                    qTb = emit_q(tb, pending)
                    if skip_attn:
                        nc.vector.tensor_copy(
                            oT[:, 0 * S + tb * TOKB: 0 * S + (tb + 1) * TOKB],
                            qTb[:, 0:TOKB])
                    else:
                        pending = attn_steps(tb, qTb)
            # wq scope closed: its SBUF range is free for wo below.
            if skip_o:
                while pending:
                    pending.pop(0)()
                return
            with tc.tile_pool(name="wo", bufs=1) as wop, \
                 tc.tile_pool(name="ys", bufs=3) as ysp, \
                 tc.tile_pool(name="py", bufs=2, space="PSUM") as pyp:
                wo_sb = wop.tile([128, NH_L * H], bf16, tag="wo")
                # hb-sliced loads (Pool queue), hb-major so early O groups
                # have their slice ready.
                for hb in range(H // 512):
                    for h in range(NH_L):
                        c0 = h * H + hb * 512
                        nc.gpsimd.dma_start(wo_sb[:, c0:c0 + 512],
                                            wo_d[:, c0:c0 + 512])

                def o_group(hb, t):
                    py = pyp.tile([128, 512], f32, tag="py")
                    for h in range(NH_L):
                        nc.tensor.matmul(
                            py[:],
                            oT[:, h * S + t * 128: h * S + (t + 1) * 128],
                            wo_sb[:, h * H + hb * 512: h * H + (hb + 1) * 512],
                            start=(h == 0), stop=(h == NH_L - 1))
                    ys = ysp.tile([128, 512], bf16, tag="ys")
                    nc.scalar.copy(ys[:], py[:])
                    nc.sync.dma_start(
                        y_d[t * 128:(t + 1) * 128, hb * 512:(hb + 1) * 512],
                        ys[:])

                # attention(3) drained under O-proj of token blocks 0..2
                early = [(hb, t) for hb in range(H // 512) for t in range(12)]
                per = len(pending) / max(1, len(early))
                acc = 0.0
                for hb, t in early:
                    o_group(hb, t)
                    acc += per
                    while acc >= 1.0 and pending:
                        pending.pop(0)()
                        acc -= 1.0
                while pending:
                    pending.pop(0)()
                for hb in range(H // 512):
                    for t in range(12, 16):
                        o_group(hb, t)

    def emit(ctx, tc):
        persist = ctx.enter_context(tc.tile_pool(name="persist", bufs=1))
        csp = ctx.enter_context(tc.tile_pool(name="cs", bufs=1))
        rp = ctx.enter_context(tc.tile_pool(name="rope", bufs=2))

        # Tiles only; DMAs are issued inside the phases in priority order.
        mask_sb = persist.tile([128, 128], bf16, tag="mask")
        id_sb = persist.tile([128, 128], bf16, tag="ident")
        cos_sb = csp.tile([128, S], f16, tag="cos")
        sinn_sb = csp.tile([128, S], f16, tag="sinn")
        kT = persist.tile([128, NKV_L * S], bf16, tag="kT")
        vA = persist.tile([128, NKV_L * NQC * VSTRIDE], bf16, tag="vA")
        nc.vector.memset(vA[:], 1.0)
        oT = persist.tile([128, NH_L * S], bf16, tag="oT")

        if phases in ("full", "kv", "kvq", "kvqa"):
            emit_kv(tc, rp, cos_sb, sinn_sb, mask_sb, id_sb, kT, vA)
        if phases == "qa":
            nc.sync.dma_start(cos_sb[:], cos_d[:])
            nc.sync.dma_start(sinn_sb[:], sin_d[:])
            nc.sync.dma_start(mask_sb[:], msk_d[:])
            nc.sync.dma_start(id_sb[:], id_d[:])
            nc.vector.memset(kT[:], 0.01)
        if phases in ("full", "qa", "kvqa", "kvq"):
            emit_qao(tc, rp, cos_sb, sinn_sb, kT, vA, mask_sb, id_sb, oT,
                     skip_attn=(phases == "kvq"),
                     skip_o=(phases in ("qa", "kvqa", "kvq")))

    with tile.TileContext(nc) as tc:
        if reps == 1:
            with ExitStack() as ctx:
                emit(ctx, tc)
        else:
            with tc.For_i(0, reps, 1):
                with ExitStack() as ctx:
                    emit(ctx, tc)
    nc.compile()
    return nc


def get_nc(reps=1):
    if reps not in _NC_CACHE:
        _NC_CACHE[reps] = _build(reps)
    return _NC_CACHE[reps]


def make_in_maps(hidden_states, position_ids, wq, wk, wv, wo):
    hidden_states = np.asarray(hidden_states, dtype=np.float32)
    position_ids = np.asarray(position_ids)
    wq = np.asarray(wq, dtype=np.float32)
    wk = np.asarray(wk, dtype=np.float32)
    wv = np.asarray(wv, dtype=np.float32)
    wo = np.asarray(wo, dtype=np.float32)

    j = np.arange(64, dtype=np.float64)
    invf = 1.0 / (THETA ** (2.0 * j / HD))       # [64]
    kp = np.arange(128)[:, None]
    qf = np.arange(TOKB)[None, :]
    maskB = (np.arange(128)[None, :] >= kp).astype(BF16)  # [128,128] triangle
    ident = np.eye(128, dtype=BF16)

    def interleave(wT, n):
        # [H_in, n] fp32 -> [128, (H_in/128)*n] bf16; row p holds all
        # contraction chunks for partition p (direct SBUF image)
        hin = wT.shape[0]
        return np.ascontiguousarray(
            wT.reshape(hin // 128, 128, n).transpose(1, 0, 2).reshape(128, -1)
        ).astype(BF16)

    in_maps = []
    for c in range(N_CORES):
        b, t = divmod(c, TP)
        pos = position_ids[b].astype(np.float64)     # [S]
        freqs = pos[:, None] * invf[None, :]         # [S, 64]
        cos64 = np.cos(freqs).astype(np.float16).T   # [64, S]
        sin64 = np.sin(freqs).astype(np.float16).T
        cosT = np.ascontiguousarray(np.concatenate([cos64, cos64], axis=0))
        sinN = np.ascontiguousarray(np.concatenate([-sin64, sin64], axis=0))

        hsT = hidden_states[b].T                     # [H, S] fp32
        # rows (tb*128 + p), cols (kc*512 + c)
        hs_i = np.ascontiguousarray(
            hsT.reshape(NKC, 128, NTB, TOKB).transpose(2, 1, 0, 3)
            .reshape(NTB * 128, NKC * TOKB)).astype(BF16)

        # wq: two contiguous half-blocks (heads 0-3, heads 4-7), each
        # interleaved [128, NKC*512] -- matches the kernel's two wq tiles.
        wq_loc = wq[t * NH_L * HD:(t + 1) * NH_L * HD, :]
        wq_i = np.concatenate(
            [interleave(np.ascontiguousarray(wq_loc[w2 * 512:(w2 + 1) * 512, :].T),
                        512) for w2 in range(2)], axis=1)
        in_maps.append({
            "hs": hs_i,
            "wq": wq_i,
            "wk": interleave(
                np.ascontiguousarray(wk[t * NKV_L * HD:(t + 1) * NKV_L * HD, :].T),
                NKV_L * HD),
            "wv": interleave(
                np.ascontiguousarray(wv[t * NKV_L * HD:(t + 1) * NKV_L * HD, :].T),
                NKV_L * HD),
            "wo": interleave(
                np.ascontiguousarray(wo[:, t * NH_L * HD:(t + 1) * NH_L * HD].T), H),
            "cosT": cosT,
            "sinN": sinN,
            "maskB": maskB,
            "ident": ident,
        })
    return in_maps


def gather_out(results):
    """results: list of 8 dicts with 'y' [S, H] bf16 -> [B, S, H] fp32."""
    out = np.zeros((B, S, H), dtype=np.float32)
    for c in range(N_CORES):
        b = c // TP
        out[b] += results[c]["y"].astype(np.float32)
    return out


def kernel(**inputs):
    from concourse.bass_utils import run_bass_kernel_spmd

    nc = get_nc(reps=1)
    in_maps = make_in_maps(**inputs)
    res = run_bass_kernel_spmd(nc, in_maps, core_ids=list(range(N_CORES)))
    return gather_out(res.results)
